# revision 1
# baseline (speedup 1.0000x reference)
"""Trainium2 Bass kernel for nn_MultiHeadAttention_61778809586301.

Head-sharded across 8 NeuronCores: core `a` computes output row-group `a`
(which, per the reference's faithful-TF recombination, is attention head `a`
across all 8 batches, concatenated batch-major along channels, then Wo+relu).

Per-core device work (all f32):
  - projections q/k/v for all 8 batches, head slice `a` (weights host-sliced,
    inputs host-transposed to feature-major so the contraction dim lands on
    SBUF partitions)
  - causal + key-mask softmax attention, exact additive-mask semantics
  - recombine via sum_c O_c @ Wo[c-slot] with relu + query-mask epilogue

Batches are processed in pairs (c, c+4) packed into SBUF partition halves so
K=64 / M=64 matmuls run as concurrent row/col-tiled pairs on the PE array.
"""
import sys

if "/opt/trn_rl_repo" not in sys.path:
    sys.path.insert(0, "/opt/trn_rl_repo")

import numpy as np

B, S, D, H, DH = 8, 1024, 512, 8, 64
NEG = np.float32(1.0e9)
NPAIR = 4          # batch pairs (p, p+4)
NBLK = S // 128    # 8 sq blocks of 128
KO = D // 128      # 4 contraction chunks of 128

_CACHE: dict = {}
RUN_KWARGS: dict = {}   # extra kwargs for run_bass_kernel_spmd (e.g. trace)
LAST_RESULT = None      # BassKernelResults of the most recent kernel() call


def _build():
    import concourse.mybir as mybir
    import concourse.tile as tile
    from concourse import bacc
    from concourse.masks import make_identity

    f32 = mybir.dt.float32
    bf16 = mybir.dt.bfloat16
    nc = bacc.Bacc(
        "TRN2",
        target_bir_lowering=False,
        debug=False,
        enable_asserts=False,
        num_devices=H,
    )

    xt_q = nc.dram_tensor("xt_q", [D, B * S], bf16, kind="ExternalInput")
    xt_k = nc.dram_tensor("xt_k", [D, B * S], bf16, kind="ExternalInput")
    xt_v = nc.dram_tensor("xt_v", [D, B * S], bf16, kind="ExternalInput")
    wq_d = nc.dram_tensor("wq", [D, DH], bf16, kind="ExternalInput")
    wk_d = nc.dram_tensor("wk", [D, DH], bf16, kind="ExternalInput")
    wv_d = nc.dram_tensor("wv", [D, DH], bf16, kind="ExternalInput")
    wo_d = nc.dram_tensor("wo_p", [NPAIR, 128, D], bf16, kind="ExternalInput")
    madd_d = nc.dram_tensor("madd", [S, S], f32, kind="ExternalInput")
    kmb_d = nc.dram_tensor("kmb", [128, S], bf16, kind="ExternalInput")
    n1t_d = nc.dram_tensor("n1t", [128, NBLK], f32, kind="ExternalInput")
    qm_d = nc.dram_tensor("qmask", [128, NBLK], f32, kind="ExternalInput")
    out_d = nc.dram_tensor("out", [S, D], f32, kind="ExternalOutput")

    with tile.TileContext(nc) as tc:
        with (
            tc.tile_pool(name="fixed", bufs=1) as fixed,
            tc.tile_pool(name="stage", bufs=3) as stage,
            tc.tile_pool(name="proj", bufs=2) as proj,
            tc.tile_pool(name="work", bufs=3) as work,
            tc.tile_pool(name="ptp", bufs=4) as ptp,
            tc.tile_pool(name="stats", bufs=6) as stats,
            tc.tile_pool(name="psA", bufs=4, space="PSUM") as psA,
            tc.tile_pool(name="psB", bufs=2, space="PSUM") as psB,
        ):
            # ---- constants / weights ----
            ident = fixed.tile([128, 128], f32, tag="ident")
            make_identity(nc, ident[:])
            ident_bf = fixed.tile([128, 128], bf16, tag="identbf")
            nc.vector.tensor_copy(ident_bf[:], ident[:])

            wq_sb = fixed.tile([128, KO, DH], bf16, tag="wq")
            wk_sb = fixed.tile([128, KO, DH], bf16, tag="wk")
            wv_sb = fixed.tile([128, KO, DH], bf16, tag="wv")
            nc.sync.dma_start(wq_sb[:], wq_d.rearrange("(ko ki) m -> ki ko m", ki=128))
            nc.sync.dma_start(wk_sb[:], wk_d.rearrange("(ko ki) m -> ki ko m", ki=128))
            nc.sync.dma_start(wv_sb[:], wv_d.rearrange("(ko ki) m -> ki ko m", ki=128))

            wo_sb = fixed.tile([128, NPAIR, D], bf16, tag="wo")
            nc.sync.dma_start(wo_sb[:], wo_d.rearrange("p ki n -> ki p n"))

            qm_sb = fixed.tile([128, NBLK], f32, tag="qm")
            nc.sync.dma_start(qm_sb[:], qm_d[:, :])

            kmb_sb = fixed.tile([128, S], bf16, tag="kmb")
            nc.sync.dma_start(kmb_sb[:], kmb_d[:, :])
            n1t_sb = fixed.tile([128, NBLK], f32, tag="n1t")
            nc.sync.dma_start(n1t_sb[:], n1t_d[:, :])
            neg_big = fixed.tile([128, 1], f32, tag="negbig")
            nc.vector.memset(neg_big[:], -1.0e9)

            madd_sb = []
            for i in range(NBLK):
                L = 128 * (i + 1)
                t = fixed.tile([128, L], f32, tag=f"madd{i}")
                nc.sync.dma_start(t[:], madd_d[128 * i:128 * (i + 1), :L])
                madd_sb.append(t)

            # persistent attention outputs, transposed: [dh of c | dh of c+4] x S
            ot_sb = [
                fixed.tile([128, S], bf16, tag=f"ot{p}", name=f"ot{p}")
                for p in range(NPAIR)
            ]

            def emit_proj(p):
                """Projections + masked-V suffix sums + V-natural for pair p."""
                qkv_pair = []
                for name, xt, w_sb in (
                    ("q", xt_q, wq_sb), ("k", xt_k, wk_sb), ("v", xt_v, wv_sb),
                ):
                    pair_t = proj.tile([128, S], bf16, tag=f"{name}T",
                                       name=f"{name}T{p}")
                    for g in range(2):
                        c = p + 4 * g
                        st = stage.tile([128, KO, S], bf16, tag="xstage",
                                        name=f"st{p}{name}{g}")
                        nc.sync.dma_start(
                            st[:],
                            xt[:, c * S:(c + 1) * S].rearrange(
                                "(ko ki) s -> ki ko s", ki=128
                            ),
                        )
                        for hh in range(2):
                            ps = psA.tile([128, 512], f32, tag="ps",
                                          name=f"psp{p}{name}{g}{hh}")
                            for ko in range(KO):
                                nc.tensor.matmul(
                                    ps[64 * g:64 * (g + 1), :],
                                    lhsT=w_sb[:, ko, :],
                                    rhs=st[:, ko, 512 * hh:512 * (hh + 1)],
                                    start=(ko == 0),
                                    stop=(ko == KO - 1),
                                )
                            nc.vector.tensor_copy(
                                pair_t[64 * g:64 * (g + 1), 512 * hh:512 * (hh + 1)],
                                ps[64 * g:64 * (g + 1), :],
                            )
                    qkv_pair.append(pair_t)
                qT, kT, vT = qkv_pair

                # masked-V suffix sums: vks[:, i] = sum_{j>=128(i+1), km=1} v[j]
                vks = proj.tile([128, NBLK], f32, tag="vks", name=f"vks{p}")
                bsum = proj.tile([128, NBLK], f32, tag="bsum", name=f"bsum{p}")
                for b in range(NBLK):
                    ttscr = work.tile([128, 128], f32, tag="ttscr",
                                      name=f"tts{p}{b}")
                    nc.vector.tensor_tensor(
                        ttscr[:],
                        vT[:, 128 * b:128 * (b + 1)],
                        kmb_sb[:, 128 * b:128 * (b + 1)],
                        mybir.AluOpType.mult,
                    )
                    nc.vector.tensor_reduce(
                        bsum[:, b:b + 1],
                        ttscr[:],
                        axis=mybir.AxisListType.X,
                        op=mybir.AluOpType.add,
                    )
                nc.vector.memset(vks[:, NBLK - 1:NBLK], 0.0)
                for b in range(NBLK - 2, -1, -1):
                    nc.vector.tensor_tensor(
                        vks[:, b:b + 1],
                        vks[:, b + 1:b + 2],
                        bsum[:, b + 1:b + 2],
                        mybir.AluOpType.add,
                    )

                # V back to natural layout [sk, dh] per (block j, g)
                vnat = proj.tile([128, NBLK, 2, DH], bf16, tag="vnat",
                                 name=f"vnat{p}")
                for g in range(2):
                    for j in range(NBLK):
                        nc.sync.dma_start_transpose(
                            vnat[:, j, g, :],
                            vT[64 * g:64 * (g + 1), 128 * j:128 * (j + 1)],
                        )
                return qT, kT, vT, vks, vnat

            def emit_attn(p, tiles):
                qT, kT, vT, vks, vnat = tiles
                for i in range(NBLK):
                    for g in range(2):
                        L = 128 * (i + 1)
                        nch = (L + 511) // 512
                        t_sb = work.tile([128, S], f32, tag="t")
                        for n in range(nch):
                            Ln = min(512, L - 512 * n)
                            ps = psA.tile([128, 512], f32, tag="ps")
                            nc.tensor.matmul(
                                ps[:, :Ln],
                                lhsT=qT[64 * g:64 * (g + 1),
                                        128 * i:128 * (i + 1)],
                                rhs=kT[64 * g:64 * (g + 1),
                                       512 * n:512 * n + Ln],
                                start=True,
                                stop=True,
                            )
                            # t = min(sims, mclip): masked entries become the
                            # exact post-mask constants (-1e9/-2e9, matching
                            # jax's f32 absorption of "x - 1e9"), unmasked pass
                            # through (mclip=+FLT_MAX). Bit-exact tie semantics
                            # independent of engine rounding modes.
                            nc.vector.tensor_tensor(
                                t_sb[:, 512 * n:512 * n + Ln],
                                ps[:, :Ln],
                                madd_sb[i][:, 512 * n:512 * n + Ln],
                                mybir.AluOpType.min,
                            )
                        mneg = stats.tile([128, 1], f32, tag="mneg")
                        nc.vector.tensor_reduce(
                            mneg[:],
                            t_sb[:, :L],
                            axis=mybir.AxisListType.X,
                            op=mybir.AluOpType.max,
                            negate=True,
                        )
                        p_sb = work.tile([128, S], f32, tag="p")
                        ssum = stats.tile([128, 1], f32, tag="ssum")
                        nc.scalar.activation(
                            p_sb[:, :L],
                            t_sb[:, :L],
                            mybir.ActivationFunctionType.Exp,
                            bias=mneg[:],
                            scale=1.0,
                            accum_out=ssum[:],
                        )
                        # virtual tail column: weight exp(-1e9 - m) per row
                        # (0 for normal rows; 1 for fully-masked rows), with
                        # n1t tail-tie count folded into the softmax sum.
                        etail = stats.tile([128, 1], f32, tag="etail")
                        nc.scalar.activation(
                            etail[:],
                            neg_big[:],
                            mybir.ActivationFunctionType.Exp,
                            bias=mneg[:],
                            scale=1.0,
                        )
                        etn = stats.tile([128, 1], f32, tag="etn")
                        nc.vector.tensor_tensor(
                            etn[:], etail[:], n1t_sb[:, i:i + 1],
                            mybir.AluOpType.mult,
                        )
                        ssum2 = stats.tile([128, 1], f32, tag="ssum2")
                        nc.vector.tensor_tensor(
                            ssum2[:], ssum[:], etn[:], mybir.AluOpType.add,
                        )
                        rcp = stats.tile([128, 1], f32, tag="rcp")
                        nc.vector.reciprocal(rcp[:], ssum2[:])
                        ptail = stats.tile([128, 1], f32, tag="ptail")
                        nc.vector.tensor_tensor(
                            ptail[:], etail[:], rcp[:], mybir.AluOpType.mult,
                        )
                        ptrep = stats.tile([128, DH], f32, tag="ptrep")
                        nc.vector.tensor_copy(
                            ptrep[:], ptail[:, 0:1].to_broadcast((128, DH)),
                        )
                        pn_sb = work.tile([128, S], bf16, tag="pn")
                        nc.scalar.activation(
                            pn_sb[:, :L],
                            p_sb[:, :L],
                            mybir.ActivationFunctionType.Identity,
                            bias=0.0,
                            scale=rcp[:],
                        )
                        # transpose P blocks and accumulate PV into OT
                        po = psB.tile([128, 128], f32, tag="pvo")
                        for j in range(i + 1):
                            pt_sb = ptp.tile([128, 128], bf16, tag="pt")
                            nc.sync.dma_start_transpose(
                                pt_sb[:],
                                pn_sb[:, 128 * j:128 * (j + 1)],
                            )
                            nc.tensor.matmul(
                                po[64 * g:64 * (g + 1), :],
                                lhsT=vnat[:, j, g, :],
                                rhs=pt_sb[:],
                                start=(j == 0),
                                stop=(j == i),
                            )
                        # tail contribution: ot = po + ptail[sq] * vks[d]
                        gs = slice(64 * g, 64 * (g + 1))
                        btail = psB.tile([128, 128], f32, tag="pb")
                        nc.tensor.matmul(
                            btail[gs, :],
                            lhsT=ptrep[:],
                            rhs=ident[:],
                            start=True,
                            stop=True,
                        )
                        ttl = work.tile([128, 128], f32, tag="ttscr")
                        nc.vector.tensor_tensor(
                            ttl[gs, :],
                            btail[gs, :],
                            vks[gs, i:i + 1].to_broadcast((64, 128)),
                            mybir.AluOpType.mult,
                        )
                        nc.vector.tensor_tensor(
                            ot_sb[p][gs, 128 * i:128 * (i + 1)],
                            po[gs, :],
                            ttl[gs, :],
                            mybir.AluOpType.add,
                        )

            # ---- software-pipelined emission: proj one pair ahead ----
            tiles = emit_proj(0)
            nxt = emit_proj(1)
            emit_attn(0, tiles)
            tiles, nxt = nxt, emit_proj(2)
            emit_attn(1, tiles)
            tiles, nxt = nxt, emit_proj(3)
            emit_attn(2, tiles)
            emit_attn(3, nxt)

            # ---- final projection + relu + query-mask ----
            for i in range(NBLK):
                ps = psA.tile([128, 512], f32, tag="ps", name=f"psf{i}")
                for p in range(NPAIR):
                    nc.tensor.matmul(
                        ps[:],
                        lhsT=ot_sb[p][:, 128 * i:128 * (i + 1)],
                        rhs=wo_sb[:, p, :],
                        start=(p == 0),
                        stop=(p == NPAIR - 1),
                    )
                o_sb = work.tile([128, D], f32, tag="osb")
                nc.scalar.activation(
                    o_sb[:],
                    ps[:],
                    mybir.ActivationFunctionType.Relu,
                    bias=0.0,
                    scale=qm_sb[:, i:i + 1],
                )
                nc.sync.dma_start(out_d[128 * i:128 * (i + 1), :], o_sb[:])

    nc.compile()
    return nc


def _get_nc():
    if "nc" not in _CACHE:
        _CACHE["nc"] = _build()
    return _CACHE["nc"]


def _host_prep(query, key, value, query_mask, key_mask, Wq, Wk, Wv, Wo):
    """Build the 8 per-core input maps (numpy only)."""
    inv = np.float32(1.0) / np.sqrt(np.float32(D))

    import ml_dtypes

    def tfeat(x):  # (B,S,D) -> feature-major (D, B*S), contiguous bf16
        return np.ascontiguousarray(
            x.reshape(B * S, D).astype(np.float32, copy=False).T
        ).astype(ml_dtypes.bfloat16)

    xq, xk, xv = tfeat(query), tfeat(key), tfeat(value)
    kmf = key_mask.astype(np.float32)
    qmf = query_mask.astype(np.float32)
    causal = np.triu(np.full((S, S), NEG, np.float32), k=1)
    Wqf = Wq.astype(np.float32, copy=False)
    Wkf = Wk.astype(np.float32, copy=False)
    Wvf = Wv.astype(np.float32, copy=False)
    Wof = Wo.astype(np.float32, copy=False)

    wo_p = np.stack(
        [
            np.concatenate(
                [Wof[p * DH:(p + 1) * DH, :], Wof[(p + 4) * DH:(p + 5) * DH, :]],
                axis=0,
            )
            for p in range(NPAIR)
        ]
    ).astype(ml_dtypes.bfloat16)  # (4, 128, 512)

    FBIG = np.finfo(np.float32).max
    in_maps = []
    for a in range(H):
        madd_sum = (causal + NEG * (1.0 - kmf[a])[None, :]).astype(np.float32)
        # min-clip tile: exact masked values where masked, +FLT_MAX where not
        madd = np.where(madd_sum > 0, -madd_sum, FBIG).astype(np.float32)
        in_maps.append(
            {
                "xt_q": xq,
                "xt_k": xk,
                "xt_v": xv,
                "wq": np.ascontiguousarray(
                    Wqf[:, a * DH:(a + 1) * DH] * inv
                ).astype(ml_dtypes.bfloat16),
                "wk": np.ascontiguousarray(
                    Wkf[:, a * DH:(a + 1) * DH]
                ).astype(ml_dtypes.bfloat16),
                "wv": np.ascontiguousarray(
                    Wvf[:, a * DH:(a + 1) * DH]
                ).astype(ml_dtypes.bfloat16),
                "wo_p": wo_p,
                "madd": madd,
                "kmb": np.ascontiguousarray(
                    np.broadcast_to(kmf[a][None, :], (128, S))
                ).astype(ml_dtypes.bfloat16),
                "n1t": np.ascontiguousarray(
                    np.broadcast_to(
                        np.array(
                            [kmf[a, 128 * (i + 1):].sum() for i in range(NBLK)],
                            np.float32,
                        )[None, :],
                        (128, NBLK),
                    )
                ),
                "qmask": np.ascontiguousarray(
                    qmf[a].reshape(NBLK, 128).T
                ),  # [p, blk] = qm[a, 128*blk + p]
                "out": None,  # placeholder removed below
            }
        )
        del in_maps[-1]["out"]
    return in_maps


def kernel(**inputs) -> np.ndarray:
    from concourse.bass_utils import run_bass_kernel_spmd

    nc = _get_nc()
    in_maps = _host_prep(
        np.asarray(inputs["query"]),
        np.asarray(inputs["key"]),
        np.asarray(inputs["value"]),
        np.asarray(inputs["query_mask"]),
        np.asarray(inputs["key_mask"]),
        np.asarray(inputs["Wq"]),
        np.asarray(inputs["Wk"]),
        np.asarray(inputs["Wv"]),
        np.asarray(inputs["Wo"]),
    )
    res = run_bass_kernel_spmd(nc, in_maps, core_ids=list(range(H)), **RUN_KWARGS)
    global LAST_RESULT
    LAST_RESULT = res
    return np.stack([res.results[a]["out"] for a in range(H)])



# revision 22
# speedup vs baseline: 1.7738x; 1.7738x over previous
"""Trainium2 Bass kernel for nn_MultiHeadAttention_61778809586301.

Head-sharded across 8 NeuronCores: core `a` computes output row-group `a`
(which, per the reference's faithful-TF recombination, is attention head `a`
across all 8 batches, concatenated batch-major along channels, then Wo+relu).

v1.5: baseline structure, but every DMA transpose (was 422us serialized on
the Sync engine / DMA XBAR) replaced with a PE-array transpose (matmul in
is_transpose mode) plus a PSUM->SBUF copy.

Per-core device work (all f32):
  - projections q/k/v for all 8 batches, head slice `a` (weights host-sliced,
    inputs host-transposed to feature-major so the contraction dim lands on
    SBUF partitions)
  - causal + key-mask softmax attention, exact additive-mask semantics
  - recombine via sum_c O_c @ Wo[c-slot] with relu + query-mask epilogue

Batches are processed in pairs (c, c+4) packed into SBUF partition halves so
K=64 / M=64 matmuls run as concurrent row/col-tiled pairs on the PE array.
"""
import sys

if "/opt/trn_rl_repo" not in sys.path:
    sys.path.insert(0, "/opt/trn_rl_repo")

import numpy as np

B, S, D, H, DH = 8, 1024, 512, 8, 64
NEG = np.float32(1.0e9)
NPAIR = 4          # batch pairs (p, p+4)
NBLK = S // 128    # 8 sq blocks of 128
KO = D // 128      # 4 contraction chunks of 128

_CACHE: dict = {}
RUN_KWARGS: dict = {}   # extra kwargs for run_bass_kernel_spmd (e.g. trace)
LAST_RESULT = None      # BassKernelResults of the most recent kernel() call


def _build():
    import concourse.mybir as mybir
    import concourse.tile as tile
    from concourse import bacc
    from concourse.masks import make_identity

    f32 = mybir.dt.float32
    bf16 = mybir.dt.bfloat16
    nc = bacc.Bacc(
        "TRN2",
        target_bir_lowering=False,
        debug=False,
        enable_asserts=False,
        num_devices=H,
    )

    xt_q = nc.dram_tensor("xt_q", [D, B * S], bf16, kind="ExternalInput")
    xt_k = nc.dram_tensor("xt_k", [D, B * S], bf16, kind="ExternalInput")
    xt_v = nc.dram_tensor("xt_v", [D, B * S], bf16, kind="ExternalInput")
    wq_d = nc.dram_tensor("wq", [D, DH], bf16, kind="ExternalInput")
    wk_d = nc.dram_tensor("wk", [D, DH], bf16, kind="ExternalInput")
    wv_d = nc.dram_tensor("wv", [D, DH], bf16, kind="ExternalInput")
    wo_d = nc.dram_tensor("wo_p", [NPAIR, 128, D], bf16, kind="ExternalInput")
    madd_d = nc.dram_tensor("madd", [S, S], f32, kind="ExternalInput")
    kmb_d = nc.dram_tensor("kmb", [128, S], bf16, kind="ExternalInput")
    n1t_d = nc.dram_tensor("n1t", [128, NBLK], f32, kind="ExternalInput")
    qm_d = nc.dram_tensor("qmask", [128, NBLK], f32, kind="ExternalInput")
    out_d = nc.dram_tensor("out", [S, D], f32, kind="ExternalOutput")

    with tile.TileContext(nc) as tc:
        with (
            tc.tile_pool(name="fixed", bufs=1) as fixed,
            tc.tile_pool(name="stage", bufs=3) as stage,
            tc.tile_pool(name="proj", bufs=2) as proj,
            tc.tile_pool(name="work", bufs=3) as work,
            tc.tile_pool(name="ptp", bufs=4) as ptp,
            tc.tile_pool(name="stats", bufs=6) as stats,
            tc.tile_pool(name="psA", bufs=3, space="PSUM") as psA,
            tc.tile_pool(name="psB", bufs=2, space="PSUM") as psB,
            tc.tile_pool(name="psT", bufs=2, space="PSUM") as psT,
        ):
            # ---- constants / weights ----
            ident = fixed.tile([128, 128], f32, tag="ident")
            make_identity(nc, ident[:])
            ident_bf = fixed.tile([128, 128], bf16, tag="identbf")
            nc.vector.tensor_copy(ident_bf[:], ident[:])

            wq_sb = fixed.tile([128, KO, DH], bf16, tag="wq")
            wk_sb = fixed.tile([128, KO, DH], bf16, tag="wk")
            wv_sb = fixed.tile([128, KO, DH], bf16, tag="wv")
            nc.sync.dma_start(wq_sb[:], wq_d.rearrange("(ko ki) m -> ki ko m", ki=128))
            nc.sync.dma_start(wk_sb[:], wk_d.rearrange("(ko ki) m -> ki ko m", ki=128))
            nc.sync.dma_start(wv_sb[:], wv_d.rearrange("(ko ki) m -> ki ko m", ki=128))

            wo_sb = fixed.tile([128, NPAIR, D], bf16, tag="wo")
            nc.sync.dma_start(wo_sb[:], wo_d.rearrange("p ki n -> ki p n"))

            qm_sb = fixed.tile([128, NBLK], f32, tag="qm")
            nc.sync.dma_start(qm_sb[:], qm_d[:, :])

            kmb_sb = fixed.tile([128, S], bf16, tag="kmb")
            nc.sync.dma_start(kmb_sb[:], kmb_d[:, :])
            n1t_sb = fixed.tile([128, NBLK], f32, tag="n1t")
            nc.sync.dma_start(n1t_sb[:], n1t_d[:, :])
            neg_big = fixed.tile([128, 1], f32, tag="negbig")
            nc.vector.memset(neg_big[:], -1.0e9)

            madd_sb = []
            for i in range(NBLK):
                L = 128 * (i + 1)
                t = fixed.tile([128, L], f32, tag=f"madd{i}")
                nc.sync.dma_start(t[:], madd_d[128 * i:128 * (i + 1), :L])
                madd_sb.append(t)

            # persistent attention outputs, transposed: [dh of c | dh of c+4] x S
            ot_sb = [
                fixed.tile([128, S], bf16, tag=f"ot{p}", name=f"ot{p}")
                for p in range(NPAIR)
            ]

            def emit_proj(p):
                """Projections + masked-V suffix sums + V-natural for pair p."""
                qkv_pair = []
                for name, xt, w_sb in (
                    ("q", xt_q, wq_sb), ("k", xt_k, wk_sb), ("v", xt_v, wv_sb),
                ):
                    pair_t = proj.tile([128, S], bf16, tag=f"{name}T",
                                       name=f"{name}T{p}")
                    for g in range(2):
                        c = p + 4 * g
                        st = stage.tile([128, KO, S], bf16, tag="xstage",
                                        name=f"st{p}{name}{g}")
                        nc.sync.dma_start(
                            st[:],
                            xt[:, c * S:(c + 1) * S].rearrange(
                                "(ko ki) s -> ki ko s", ki=128
                            ),
                        )
                        for hh in range(2):
                            ps = psA.tile([128, 512], f32, tag="ps",
                                          name=f"psp{p}{name}{g}{hh}")
                            for ko in range(KO):
                                nc.tensor.matmul(
                                    ps[64 * g:64 * (g + 1), :],
                                    lhsT=w_sb[:, ko, :],
                                    rhs=st[:, ko, 512 * hh:512 * (hh + 1)],
                                    start=(ko == 0),
                                    stop=(ko == KO - 1),
                                )
                            nc.vector.tensor_copy(
                                pair_t[64 * g:64 * (g + 1), 512 * hh:512 * (hh + 1)],
                                ps[64 * g:64 * (g + 1), :],
                            )
                    qkv_pair.append(pair_t)
                qT, kT, vT = qkv_pair

                # masked-V suffix sums: vks[:, i] = sum_{j>=128(i+1), km=1} v[j]
                vks = proj.tile([128, NBLK], f32, tag="vks", name=f"vks{p}")
                bsum = proj.tile([128, NBLK], f32, tag="bsum", name=f"bsum{p}")
                for b in range(NBLK):
                    ttscr = work.tile([128, 128], f32, tag="ttscr",
                                      name=f"tts{p}{b}")
                    nc.vector.tensor_tensor(
                        ttscr[:],
                        vT[:, 128 * b:128 * (b + 1)],
                        kmb_sb[:, 128 * b:128 * (b + 1)],
                        mybir.AluOpType.mult,
                    )
                    nc.vector.tensor_reduce(
                        bsum[:, b:b + 1],
                        ttscr[:],
                        axis=mybir.AxisListType.X,
                        op=mybir.AluOpType.add,
                    )
                nc.vector.memset(vks[:, NBLK - 1:NBLK], 0.0)
                for b in range(NBLK - 2, -1, -1):
                    nc.vector.tensor_tensor(
                        vks[:, b:b + 1],
                        vks[:, b + 1:b + 2],
                        bsum[:, b + 1:b + 2],
                        mybir.AluOpType.add,
                    )

                # V back to natural layout [sk, dh] per (block j, g) via PE
                # transpose. One PSUM batch per g: the PE tile config (base
                # quadrant) cannot change inside an accumulation group.
                vnat = proj.tile([128, 2, NBLK, DH], bf16, tag="vnat",
                                 name=f"vnat{p}")
                for g in range(2):
                    psv = psT.tile([128, 1024], bf16, tag="tp1024",
                                   name=f"psv{p}{g}")
                    for j in range(NBLK):
                        nc.tensor.matmul(
                            psv[:, 64 * j:64 * (j + 1)],
                            lhsT=vT[64 * g:64 * (g + 1),
                                    128 * j:128 * (j + 1)],
                            rhs=ident_bf[64 * g:64 * (g + 1),
                                         64 * g:64 * (g + 1)],
                            is_transpose=True,
                            start=(j == 0),
                            stop=(j == NBLK - 1),
                        )
                    nc.scalar.activation(
                        vnat[:, g],
                        psv[:, 0:512],
                        mybir.ActivationFunctionType.Identity,
                        bias=0.0,
                        scale=1.0,
                    )
                return qT, kT, vT, vks, vnat

            def emit_attn(p, tiles):
                qT, kT, vT, vks, vnat = tiles
                for i in range(NBLK):
                    for g in range(2):
                        L = 128 * (i + 1)
                        nch = (L + 511) // 512
                        t_sb = work.tile([128, S], f32, tag="t")
                        for n in range(nch):
                            Ln = min(512, L - 512 * n)
                            ps = psA.tile([128, 512], f32, tag="ps")
                            nc.tensor.matmul(
                                ps[:, :Ln],
                                lhsT=qT[64 * g:64 * (g + 1),
                                        128 * i:128 * (i + 1)],
                                rhs=kT[64 * g:64 * (g + 1),
                                       512 * n:512 * n + Ln],
                                start=True,
                                stop=True,
                            )
                            # t = min(sims, mclip): masked entries become the
                            # exact post-mask constants (-1e9/-2e9, matching
                            # jax's f32 absorption of "x - 1e9"), unmasked pass
                            # through (mclip=+FLT_MAX). Bit-exact tie semantics
                            # independent of engine rounding modes.
                            nc.vector.tensor_tensor(
                                t_sb[:, 512 * n:512 * n + Ln],
                                ps[:, :Ln],
                                madd_sb[i][:, 512 * n:512 * n + Ln],
                                mybir.AluOpType.min,
                            )
                        mneg = stats.tile([128, 1], f32, tag="mneg")
                        nc.vector.tensor_reduce(
                            mneg[:],
                            t_sb[:, :L],
                            axis=mybir.AxisListType.X,
                            op=mybir.AluOpType.max,
                            negate=True,
                        )
                        p_sb = work.tile([128, S], f32, tag="p")
                        ssum = stats.tile([128, 1], f32, tag="ssum")
                        nc.scalar.activation(
                            p_sb[:, :L],
                            t_sb[:, :L],
                            mybir.ActivationFunctionType.Exp,
                            bias=mneg[:],
                            scale=1.0,
                            accum_out=ssum[:],
                        )
                        # virtual tail column: weight exp(-1e9 - m) per row
                        # (0 for normal rows; 1 for fully-masked rows), with
                        # n1t tail-tie count folded into the softmax sum.
                        etail = stats.tile([128, 1], f32, tag="etail")
                        nc.scalar.activation(
                            etail[:],
                            neg_big[:],
                            mybir.ActivationFunctionType.Exp,
                            bias=mneg[:],
                            scale=1.0,
                        )
                        etn = stats.tile([128, 1], f32, tag="etn")
                        nc.vector.tensor_tensor(
                            etn[:], etail[:], n1t_sb[:, i:i + 1],
                            mybir.AluOpType.mult,
                        )
                        ssum2 = stats.tile([128, 1], f32, tag="ssum2")
                        nc.vector.tensor_tensor(
                            ssum2[:], ssum[:], etn[:], mybir.AluOpType.add,
                        )
                        rcp = stats.tile([128, 1], f32, tag="rcp")
                        nc.vector.reciprocal(rcp[:], ssum2[:])
                        ptail = stats.tile([128, 1], f32, tag="ptail")
                        nc.vector.tensor_tensor(
                            ptail[:], etail[:], rcp[:], mybir.AluOpType.mult,
                        )
                        ptrep = stats.tile([128, DH], f32, tag="ptrep")
                        nc.vector.tensor_copy(
                            ptrep[:], ptail[:, 0:1].to_broadcast((128, DH)),
                        )
                        pn_sb = work.tile([128, S], bf16, tag="pn")
                        nc.scalar.activation(
                            pn_sb[:, :L],
                            p_sb[:, :L],
                            mybir.ActivationFunctionType.Identity,
                            bias=0.0,
                            scale=rcp[:],
                        )
                        # transpose P blocks on the PE array (batched into one
                        # PSUM bank), then accumulate PV into OT
                        pst = psT.tile([128, 1024], bf16, tag="tp1024")
                        for j in range(i + 1):
                            nc.tensor.matmul(
                                pst[:, 128 * j:128 * (j + 1)],
                                lhsT=pn_sb[:, 128 * j:128 * (j + 1)],
                                rhs=ident_bf[:],
                                is_transpose=True,
                                start=(j == 0),
                                stop=(j == i),
                            )
                        po = psB.tile([128, 128], f32, tag="pvb")
                        for j in range(i + 1):
                            pt_sb = ptp.tile([128, 128], bf16, tag="pt")
                            if (i + j) % 2 == 0:
                                nc.vector.tensor_copy(
                                    pt_sb[:], pst[:, 128 * j:128 * (j + 1)],
                                )
                            else:
                                nc.scalar.activation(
                                    pt_sb[:], pst[:, 128 * j:128 * (j + 1)],
                                    mybir.ActivationFunctionType.Identity,
                                    bias=0.0, scale=1.0,
                                )
                            nc.tensor.matmul(
                                po[64 * g:64 * (g + 1), :],
                                lhsT=vnat[:, g, j, :],
                                rhs=pt_sb[:],
                                start=(j == 0),
                                stop=(j == i),
                            )
                        # tail contribution: ot = po + ptail[sq] * vks[d]
                        gs = slice(64 * g, 64 * (g + 1))
                        btail = psB.tile([128, 128], f32, tag="pvb")
                        nc.tensor.matmul(
                            btail[gs, :],
                            lhsT=ptrep[:],
                            rhs=ident[:],
                            start=True,
                            stop=True,
                        )
                        ttl = work.tile([128, 128], f32, tag="ttscr")
                        nc.vector.tensor_tensor(
                            ttl[gs, :],
                            btail[gs, :],
                            vks[gs, i:i + 1].to_broadcast((64, 128)),
                            mybir.AluOpType.mult,
                        )
                        nc.vector.tensor_tensor(
                            ot_sb[p][gs, 128 * i:128 * (i + 1)],
                            po[gs, :],
                            ttl[gs, :],
                            mybir.AluOpType.add,
                        )

            # ---- software-pipelined emission: proj one pair ahead ----
            tiles = emit_proj(0)
            nxt = emit_proj(1)
            emit_attn(0, tiles)
            tiles, nxt = nxt, emit_proj(2)
            emit_attn(1, tiles)
            tiles, nxt = nxt, emit_proj(3)
            emit_attn(2, tiles)
            emit_attn(3, nxt)

            # ---- final projection + relu + query-mask ----
            for i in range(NBLK):
                ps = psA.tile([128, 512], f32, tag="ps", name=f"psf{i}")
                for p in range(NPAIR):
                    nc.tensor.matmul(
                        ps[:],
                        lhsT=ot_sb[p][:, 128 * i:128 * (i + 1)],
                        rhs=wo_sb[:, p, :],
                        start=(p == 0),
                        stop=(p == NPAIR - 1),
                    )
                o_sb = work.tile([128, D], f32, tag="osb")
                nc.scalar.activation(
                    o_sb[:],
                    ps[:],
                    mybir.ActivationFunctionType.Relu,
                    bias=0.0,
                    scale=qm_sb[:, i:i + 1],
                )
                nc.sync.dma_start(out_d[128 * i:128 * (i + 1), :], o_sb[:])

    nc.compile()
    return nc


def _get_nc():
    if "nc" not in _CACHE:
        _CACHE["nc"] = _build()
    return _CACHE["nc"]


def _host_prep(query, key, value, query_mask, key_mask, Wq, Wk, Wv, Wo):
    """Build the 8 per-core input maps (numpy only)."""
    inv = np.float32(1.0) / np.sqrt(np.float32(D))

    import ml_dtypes

    def tfeat(x):  # (B,S,D) -> feature-major (D, B*S), contiguous bf16
        return np.ascontiguousarray(
            x.reshape(B * S, D).astype(np.float32, copy=False).T
        ).astype(ml_dtypes.bfloat16)

    xq, xk, xv = tfeat(query), tfeat(key), tfeat(value)
    kmf = key_mask.astype(np.float32)
    qmf = query_mask.astype(np.float32)
    causal = np.triu(np.full((S, S), NEG, np.float32), k=1)
    Wqf = Wq.astype(np.float32, copy=False)
    Wkf = Wk.astype(np.float32, copy=False)
    Wvf = Wv.astype(np.float32, copy=False)
    Wof = Wo.astype(np.float32, copy=False)

    wo_p = np.stack(
        [
            np.concatenate(
                [Wof[p * DH:(p + 1) * DH, :], Wof[(p + 4) * DH:(p + 5) * DH, :]],
                axis=0,
            )
            for p in range(NPAIR)
        ]
    ).astype(ml_dtypes.bfloat16)  # (4, 128, 512)

    FBIG = np.finfo(np.float32).max
    in_maps = []
    for a in range(H):
        madd_sum = (causal + NEG * (1.0 - kmf[a])[None, :]).astype(np.float32)
        # min-clip tile: exact masked values where masked, +FLT_MAX where not
        madd = np.where(madd_sum > 0, -madd_sum, FBIG).astype(np.float32)
        in_maps.append(
            {
                "xt_q": xq,
                "xt_k": xk,
                "xt_v": xv,
                "wq": np.ascontiguousarray(
                    Wqf[:, a * DH:(a + 1) * DH] * inv
                ).astype(ml_dtypes.bfloat16),
                "wk": np.ascontiguousarray(
                    Wkf[:, a * DH:(a + 1) * DH]
                ).astype(ml_dtypes.bfloat16),
                "wv": np.ascontiguousarray(
                    Wvf[:, a * DH:(a + 1) * DH]
                ).astype(ml_dtypes.bfloat16),
                "wo_p": wo_p,
                "madd": madd,
                "kmb": np.ascontiguousarray(
                    np.broadcast_to(kmf[a][None, :], (128, S))
                ).astype(ml_dtypes.bfloat16),
                "n1t": np.ascontiguousarray(
                    np.broadcast_to(
                        np.array(
                            [kmf[a, 128 * (i + 1):].sum() for i in range(NBLK)],
                            np.float32,
                        )[None, :],
                        (128, NBLK),
                    )
                ),
                "qmask": np.ascontiguousarray(
                    qmf[a].reshape(NBLK, 128).T
                ),  # [p, blk] = qm[a, 128*blk + p]
            }
        )
    return in_maps


def kernel(**inputs) -> np.ndarray:
    from concourse.bass_utils import run_bass_kernel_spmd

    nc = _get_nc()
    in_maps = _host_prep(
        np.asarray(inputs["query"]),
        np.asarray(inputs["key"]),
        np.asarray(inputs["value"]),
        np.asarray(inputs["query_mask"]),
        np.asarray(inputs["key_mask"]),
        np.asarray(inputs["Wq"]),
        np.asarray(inputs["Wk"]),
        np.asarray(inputs["Wv"]),
        np.asarray(inputs["Wo"]),
    )
    res = run_bass_kernel_spmd(nc, in_maps, core_ids=list(range(H)), **RUN_KWARGS)
    global LAST_RESULT
    LAST_RESULT = res
    return np.stack([res.results[a]["out"] for a in range(H)])


# revision 27
# speedup vs baseline: 2.0495x; 1.1554x over previous
"""Trainium2 Bass kernel for nn_MultiHeadAttention_61778809586301.

Head-sharded across 8 NeuronCores: core `a` computes output row-group `a`
(which, per the reference's faithful-TF recombination, is attention head `a`
across all 8 batches, concatenated batch-major along channels, then Wo+relu).

v2: all transposes moved off the DMA XBAR (was 422us serialized on Sync)
onto the PE array; mask-clip+rowmax fused into one DVE op; PV matmuls
grouped by key-block j (shares LDWEIGHTS, exploits PSUM zero-region
semantics for the triangular accumulation); fully-masked-row tail applied
as rank-1 PE matmuls accumulated straight into the PV PSUM.

Per-core device work (all f32 accumulate, bf16 operands):
  - projections q/k/v for all 8 batches, head slice `a` (weights host-sliced,
    inputs host-transposed feature-major so contraction lands on partitions)
  - causal + key-mask softmax attention, exact additive-mask semantics
    (min-clip against precomputed f32-absorbed mask values)
  - recombine via sum_c O_c @ Wo[c-slot] with relu + query-mask epilogue

Batches are processed in pairs (c, c+4) packed into SBUF partition halves so
K=64 / M=64 matmuls run as concurrent row/col-tiled pairs on the PE array.
"""
import sys

if "/opt/trn_rl_repo" not in sys.path:
    sys.path.insert(0, "/opt/trn_rl_repo")

import numpy as np

B, S, D, H, DH = 8, 1024, 512, 8, 64
NEG = np.float32(1.0e9)
NPAIR = 4          # batch pairs (p, p+4)
NBLK = S // 128    # 8 sq blocks of 128
KO = D // 128      # 4 contraction chunks of 128
FBIG = float(np.finfo(np.float32).max)

_CACHE: dict = {}
RUN_KWARGS: dict = {}   # extra kwargs for run_bass_kernel_spmd (e.g. trace)
LAST_RESULT = None      # BassKernelResults of the most recent kernel() call


def _build():
    import os
    import concourse.mybir as mybir
    import concourse.tile as tile
    from concourse import bacc
    from concourse.masks import make_identity

    V_TAIL = int(os.environ.get("V_TAIL", "1"))      # rank-1 tail matmuls
    V_BATCHT = int(os.environ.get("V_BATCHT", "1"))  # batched PE transposes
    V_GROUPPV = int(os.environ.get("V_GROUPPV", "1"))  # j-grouped PV

    f32 = mybir.dt.float32
    bf16 = mybir.dt.bfloat16
    AX = mybir.AxisListType.X
    OP = mybir.AluOpType
    ACT = mybir.ActivationFunctionType
    nc = bacc.Bacc(
        "TRN2",
        target_bir_lowering=False,
        debug=False,
        enable_asserts=False,
        num_devices=H,
    )

    xt_q = nc.dram_tensor("xt_q", [D, B * S], bf16, kind="ExternalInput")
    xt_k = nc.dram_tensor("xt_k", [D, B * S], bf16, kind="ExternalInput")
    xt_v = nc.dram_tensor("xt_v", [D, B * S], bf16, kind="ExternalInput")
    wq_d = nc.dram_tensor("wq", [D, DH], bf16, kind="ExternalInput")
    wk_d = nc.dram_tensor("wk", [D, DH], bf16, kind="ExternalInput")
    wv_d = nc.dram_tensor("wv", [D, DH], bf16, kind="ExternalInput")
    wo_d = nc.dram_tensor("wo_p", [NPAIR, 128, D], bf16, kind="ExternalInput")
    madd_d = nc.dram_tensor("madd", [S, S], f32, kind="ExternalInput")
    kmb_d = nc.dram_tensor("kmb", [128, S], bf16, kind="ExternalInput")
    n1t_d = nc.dram_tensor("n1t", [128, NBLK], f32, kind="ExternalInput")
    qm_d = nc.dram_tensor("qmask", [128, NBLK], f32, kind="ExternalInput")
    out_d = nc.dram_tensor("out", [S, D], f32, kind="ExternalOutput")

    with tile.TileContext(nc) as tc:
        with (
            tc.tile_pool(name="fixed", bufs=1) as fixed,
            tc.tile_pool(name="stage", bufs=3) as stage,
            tc.tile_pool(name="proj", bufs=2) as proj,
            tc.tile_pool(name="work", bufs=3) as work,
            tc.tile_pool(name="stats", bufs=6) as stats,
            tc.tile_pool(name="psA", bufs=2, space="PSUM") as psA,
            tc.tile_pool(name="psT", bufs=3, space="PSUM") as psT,
            tc.tile_pool(name="psPV", bufs=1, space="PSUM") as psPV,
        ):
            # ---- constants / weights ----
            ident = fixed.tile([128, 128], f32, tag="ident")
            make_identity(nc, ident[:])
            ident_bf = fixed.tile([128, 128], bf16, tag="identbf")
            nc.vector.tensor_copy(ident_bf[:], ident[:])

            wq_sb = fixed.tile([128, KO, DH], bf16, tag="wq")
            wk_sb = fixed.tile([128, KO, DH], bf16, tag="wk")
            wv_sb = fixed.tile([128, KO, DH], bf16, tag="wv")
            nc.sync.dma_start(wq_sb[:], wq_d.rearrange("(ko ki) m -> ki ko m", ki=128))
            nc.sync.dma_start(wk_sb[:], wk_d.rearrange("(ko ki) m -> ki ko m", ki=128))
            nc.sync.dma_start(wv_sb[:], wv_d.rearrange("(ko ki) m -> ki ko m", ki=128))

            wo_sb = fixed.tile([128, NPAIR, D], bf16, tag="wo")
            nc.sync.dma_start(wo_sb[:], wo_d.rearrange("p ki n -> ki p n"))

            qm_sb = fixed.tile([128, NBLK], f32, tag="qm")
            nc.sync.dma_start(qm_sb[:], qm_d[:, :])

            kmb_sb = fixed.tile([128, S], bf16, tag="kmb")
            nc.sync.dma_start(kmb_sb[:], kmb_d[:, :])
            n1t_sb = fixed.tile([128, NBLK], f32, tag="n1t")
            nc.sync.dma_start(n1t_sb[:], n1t_d[:, :])
            neg_big = fixed.tile([128, 1], f32, tag="negbig")
            nc.vector.memset(neg_big[:], -1.0e9)

            madd_sb = []
            for i in range(NBLK):
                L = 128 * (i + 1)
                t = fixed.tile([128, L], f32, tag=f"madd{i}")
                nc.sync.dma_start(t[:], madd_d[128 * i:128 * (i + 1), :L])
                madd_sb.append(t)

            # persistent attention outputs, transposed: [dh of c | dh of c+4] x S
            ot_sb = [
                fixed.tile([128, S], bf16, tag=f"ot{p}", name=f"ot{p}")
                for p in range(NPAIR)
            ]
            # P^T block store: [j, i] grid of 128x128 bf16 blocks, per g.
            # Slot (j, i) holds P^T of score block (sq=i, sk=j); i<j unused.
            ptg = [
                fixed.tile([128, NBLK, NBLK, 128], bf16, tag=f"ptg{g}",
                           name=f"ptg{g}")
                for g in range(2)
            ]

            def emit_proj(p):
                """Projections + masked-V suffix sums + V-natural for pair p."""
                qkv_pair = []
                for name, xt, w_sb in (
                    ("q", xt_q, wq_sb), ("k", xt_k, wk_sb), ("v", xt_v, wv_sb),
                ):
                    pair_t = proj.tile([128, S], bf16, tag=f"{name}T",
                                       name=f"{name}T{p}")
                    for g in range(2):
                        c = p + 4 * g
                        st = stage.tile([128, KO, S], bf16, tag="xstage",
                                        name=f"st{p}{name}{g}")
                        nc.sync.dma_start(
                            st[:],
                            xt[:, c * S:(c + 1) * S].rearrange(
                                "(ko ki) s -> ki ko s", ki=128
                            ),
                        )
                        for hh in range(2):
                            ps = psA.tile([128, 512], f32, tag="ps",
                                          name=f"psp{p}{name}{g}{hh}")
                            for ko in range(KO):
                                nc.tensor.matmul(
                                    ps[64 * g:64 * (g + 1), :],
                                    lhsT=w_sb[:, ko, :],
                                    rhs=st[:, ko, 512 * hh:512 * (hh + 1)],
                                    start=(ko == 0),
                                    stop=(ko == KO - 1),
                                )
                            eng = nc.scalar if (g + hh) % 2 == 0 else None
                            if eng is None:
                                nc.vector.tensor_copy(
                                    pair_t[64 * g:64 * (g + 1),
                                           512 * hh:512 * (hh + 1)],
                                    ps[64 * g:64 * (g + 1), :],
                                )
                            else:
                                nc.scalar.activation(
                                    pair_t[64 * g:64 * (g + 1),
                                           512 * hh:512 * (hh + 1)],
                                    ps[64 * g:64 * (g + 1), :],
                                    ACT.Identity,
                                    bias=0.0,
                                    scale=1.0,
                                )
                    qkv_pair.append(pair_t)
                qT, kT, vT = qkv_pair

                # masked-V suffix sums: vks[:, i] = sum_{j>=128(i+1), km=1} v[j]
                vks = proj.tile([128, NBLK], f32, tag="vks", name=f"vks{p}")
                bsum = proj.tile([128, NBLK], f32, tag="bsum", name=f"bsum{p}")
                ttscr = work.tile([128, 128], f32, tag="ttscr",
                                  name=f"tts{p}")
                for b in range(NBLK):
                    nc.vector.scalar_tensor_tensor(
                        out=ttscr[:],
                        in0=vT[:, 128 * b:128 * (b + 1)],
                        scalar=1.0,
                        in1=kmb_sb[:, 128 * b:128 * (b + 1)],
                        op0=OP.mult,
                        op1=OP.mult,
                        accum_out=bsum[:, b:b + 1],
                    )
                nc.vector.memset(vks[:, NBLK - 1:NBLK], 0.0)
                for b in range(NBLK - 2, -1, -1):
                    nc.vector.tensor_tensor(
                        vks[:, b:b + 1], vks[:, b + 1:b + 2],
                        bsum[:, b + 1:b + 2], OP.add,
                    )
                # single-column PE transposes: vksT[0, i, :] = vks[:, i]^T,
                # everything lands on partition 0 (tail matmuls must keep a
                # fixed base quadrant)
                vksT = proj.tile([128, NBLK, 128], bf16, tag="vksT",
                                 name=f"vksT{p}")
                for half in range(2):
                    pst = psA.tile([128, 512], f32, tag="ps",
                                   name=f"vksT{p}h{half}")
                    for t in range(4):
                        i = 4 * half + t
                        nc.tensor.matmul(
                            pst[0:1, 128 * t:128 * (t + 1)],
                            lhsT=vks[:, i:i + 1],
                            rhs=ident[:],
                            is_transpose=True,
                            start=(t == 0),
                            stop=(t == 3),
                        )
                    nc.vector.tensor_copy(
                        vksT[0:1, 4 * half:4 * half + 4, :],
                        pst[0:1, 0:512],
                    )
                # V back to natural layout [sk, dh] per (block j, g) via PE
                # transpose, batched 16 blocks into one PSUM bank.
                vnat = proj.tile([128, 2, NBLK, DH], bf16, tag="vnat",
                                 name=f"vnat{p}")
                for g in range(2):
                    psv = psT.tile([128, 1024], bf16, tag="tp1024",
                                   name=f"psv{p}{g}")
                    for j in range(NBLK):
                        nc.tensor.matmul(
                            psv[:, 64 * j:64 * (j + 1)],
                            lhsT=vT[64 * g:64 * (g + 1),
                                    128 * j:128 * (j + 1)],
                            rhs=ident_bf[64 * g:64 * (g + 1),
                                         64 * g:64 * (g + 1)],
                            is_transpose=True,
                            start=(j == 0),
                            stop=(j == NBLK - 1),
                        )
                    nc.scalar.activation(
                        vnat[:, g],
                        psv[:, 0:512],
                        ACT.Identity,
                        bias=0.0,
                        scale=1.0,
                    )
                return qT, kT, vT, vksT, vnat

            def emit_attn(p, tiles):
                qT, kT, vT, vksT, vnat = tiles
                ptailT = [None, None]  # [g]
                for g in range(2):
                    ptail_all = stats.tile([128, NBLK], f32, tag="ptall",
                                           name=f"ptall{p}{g}")
                    for i in range(NBLK):
                        L = 128 * (i + 1)
                        nch = (L + 511) // 512
                        t_sb = work.tile([128, S], f32, tag="tneg")
                        for n in range(nch):
                            Ln = min(512, L - 512 * n)
                            ps = psA.tile([128, 512], f32, tag="ps")
                            nc.tensor.matmul(
                                ps[:, :Ln],
                                lhsT=qT[64 * g:64 * (g + 1),
                                        128 * i:128 * (i + 1)],
                                rhs=kT[64 * g:64 * (g + 1),
                                       512 * n:512 * n + Ln],
                                start=True,
                                stop=True,
                            )
                            # t = min(sims, mclip): masked entries become the
                            # exact post-mask constants (-1e9/-2e9, matching
                            # jax's f32 absorption of "x - 1e9").
                            nc.vector.tensor_tensor(
                                t_sb[:, 512 * n:512 * n + Ln],
                                ps[:, :Ln],
                                madd_sb[i][:, 512 * n:512 * n + Ln],
                                OP.min,
                            )
                        mneg = stats.tile([128, 1], f32, tag="mneg")
                        nc.vector.tensor_reduce(
                            mneg[:],
                            t_sb[:, :L],
                            axis=AX,
                            op=OP.max,
                            negate=True,
                        )
                        p_sb = work.tile([128, S], f32, tag="p")
                        ssum = stats.tile([128, 1], f32, tag="ssum")
                        nc.scalar.activation(
                            p_sb[:, :L],
                            t_sb[:, :L],
                            ACT.Exp,
                            bias=mneg[:],
                            scale=1.0,
                            accum_out=ssum[:],
                        )
                        # virtual tail column: weight exp(-1e9 - m) per row
                        # (0 for normal rows; 1 for fully-masked rows), with
                        # n1t tail-tie count folded into the softmax sum.
                        etail = stats.tile([128, 1], f32, tag="etail")
                        nc.scalar.activation(
                            etail[:],
                            neg_big[:],
                            ACT.Exp,
                            bias=mneg[:],
                            scale=1.0,
                        )
                        ssum2 = stats.tile([128, 1], f32, tag="ssum2")
                        nc.vector.scalar_tensor_tensor(
                            out=ssum2[:],
                            in0=etail[:],
                            scalar=n1t_sb[:, i:i + 1],
                            in1=ssum[:],
                            op0=OP.mult,
                            op1=OP.add,
                        )
                        rcp = stats.tile([128, 1], f32, tag="rcp")
                        nc.vector.reciprocal(rcp[:], ssum2[:])
                        nc.vector.tensor_tensor(
                            ptail_all[:, i:i + 1],
                            etail[:], rcp[:], OP.mult,
                        )
                        pn_sb = work.tile([128, S], bf16, tag="pn")
                        nc.scalar.activation(
                            pn_sb[:, :L],
                            p_sb[:, :L],
                            ACT.Identity,
                            bias=0.0,
                            scale=rcp[:],
                        )
                        # PE-transpose the (i+1) P blocks into one PSUM bank,
                        # then one batched copy into the [j, i] grid.
                        if V_BATCHT:
                            pst = psT.tile([128, 1024], bf16, tag="tp1024")
                            for j in range(i + 1):
                                nc.tensor.matmul(
                                    pst[:, 128 * j:128 * (j + 1)],
                                    lhsT=pn_sb[:, 128 * j:128 * (j + 1)],
                                    rhs=ident_bf[:],
                                    is_transpose=True,
                                    start=(j == 0),
                                    stop=(j == i),
                                )
                            dst = ptg[g][:, 0:i + 1, i, :]
                            if i % 2 == 0:
                                nc.vector.tensor_copy(dst, pst[:, :L])
                            else:
                                nc.scalar.activation(
                                    dst, pst[:, :L], ACT.Identity,
                                    bias=0.0, scale=1.0,
                                )
                        else:
                            for j in range(i + 1):
                                pst = psT.tile([128, 1024], bf16,
                                               tag="tp1024")
                                nc.tensor.matmul(
                                    pst[:, 0:128],
                                    lhsT=pn_sb[:, 128 * j:128 * (j + 1)],
                                    rhs=ident_bf[:],
                                    is_transpose=True,
                                    start=True,
                                    stop=True,
                                )
                                dst = ptg[g][:, j, i, :]
                                if (i + j) % 2 == 0:
                                    nc.vector.tensor_copy(dst, pst[:, 0:128])
                                else:
                                    nc.scalar.activation(
                                        dst, pst[:, 0:128], ACT.Identity,
                                        bias=0.0, scale=1.0,
                                    )
                    # single-column transposes: ptailT[0, i, :] = ptail^T,
                    # all rows at partition 0 (fixed tail quadrant)
                    ptT = stats.tile([128, NBLK, 128], bf16, tag="ptTs",
                                     name=f"ptTs{p}{g}")
                    for half in range(2):
                        pst = psA.tile([128, 512], f32, tag="ps",
                                       name=f"ptT{p}{g}h{half}")
                        for t in range(4):
                            i = 4 * half + t
                            nc.tensor.matmul(
                                pst[0:1, 128 * t:128 * (t + 1)],
                                lhsT=ptail_all[:, i:i + 1],
                                rhs=ident[:],
                                is_transpose=True,
                                start=(t == 0),
                                stop=(t == 3),
                            )
                        nc.vector.tensor_copy(
                            ptT[0:1, 4 * half:4 * half + 4, :],
                            pst[0:1, 0:512],
                        )
                    ptailT[g] = ptT

                # PV grouped by key block j: lhsT (V natural) loaded once per
                # (j, g); triangular column extents ride on PSUM zero-region
                # semantics (j==0 start zeroes the full bank).
                ps_pv = psPV.tile([128, 1024], f32, tag="pv", name=f"pv{p}")
                for g in range(2):
                    gs = slice(64 * g, 64 * (g + 1))
                    if V_GROUPPV:
                        for j in range(NBLK):
                            if j < 4:
                                nc.tensor.matmul(
                                    ps_pv[gs, 128 * j:512],
                                    lhsT=vnat[:, g, j, :],
                                    rhs=ptg[g][:, j, j:4, :],
                                    start=(j == 0),
                                    stop=(j == 3),
                                )
                                nc.tensor.matmul(
                                    ps_pv[gs, 512:1024],
                                    lhsT=vnat[:, g, j, :],
                                    rhs=ptg[g][:, j, 4:8, :],
                                    start=(j == 0),
                                    stop=False,
                                )
                            else:
                                nc.tensor.matmul(
                                    ps_pv[gs, 128 * j:1024],
                                    lhsT=vnat[:, g, j, :],
                                    rhs=ptg[g][:, j, j:8, :],
                                    start=False,
                                    stop=(j == NBLK - 1),
                                )
                    else:
                        for i in range(NBLK):
                            for j in range(i + 1):
                                nc.tensor.matmul(
                                    ps_pv[gs, 128 * i:128 * (i + 1)],
                                    lhsT=vnat[:, g, j, :],
                                    rhs=ptg[g][:, j, i, :],
                                    start=(j == 0),
                                    stop=(not V_TAIL and j == i),
                                    skip_group_check=True,
                                )
                    # tail: rank-1 ptail[sq]*vks[dh] per block, base-0 K=1
                    # matmuls continuing the closed PV accumulation (fixed
                    # quadrant; rotation inside an accumulation chain faults)
                    for i in range(NBLK):
                        nc.tensor.matmul(
                            ps_pv[gs, 128 * i:128 * (i + 1)],
                            lhsT=vksT[0:1, i, 64 * g:64 * (g + 1)],
                            rhs=ptailT[g][0:1, i, :],
                            start=False,
                            stop=True,
                            skip_group_check=True,
                        )
                    # drain attention output (normalized, tail included)
                    nc.vector.tensor_copy(
                        ot_sb[p][gs, 0:512], ps_pv[gs, 0:512],
                    )
                    nc.scalar.activation(
                        ot_sb[p][gs, 512:1024], ps_pv[gs, 512:1024],
                        ACT.Identity, bias=0.0, scale=1.0,
                    )

            # ---- software-pipelined emission: proj one pair ahead ----
            tiles = emit_proj(0)
            nxt = emit_proj(1)
            emit_attn(0, tiles)
            tiles, nxt = nxt, emit_proj(2)
            emit_attn(1, tiles)
            tiles, nxt = nxt, emit_proj(3)
            emit_attn(2, tiles)
            emit_attn(3, nxt)

            # ---- final projection + relu + query-mask ----
            for i in range(NBLK):
                ps = psA.tile([128, 512], f32, tag="ps", name=f"psf{i}")
                for p in range(NPAIR):
                    nc.tensor.matmul(
                        ps[:],
                        lhsT=ot_sb[p][:, 128 * i:128 * (i + 1)],
                        rhs=wo_sb[:, p, :],
                        start=(p == 0),
                        stop=(p == NPAIR - 1),
                    )
                o_sb = work.tile([128, D], f32, tag="osb")
                nc.scalar.activation(
                    o_sb[:],
                    ps[:],
                    ACT.Relu,
                    bias=0.0,
                    scale=qm_sb[:, i:i + 1],
                )
                nc.sync.dma_start(out_d[128 * i:128 * (i + 1), :], o_sb[:])

    nc.compile()
    return nc


def _get_nc():
    if "nc" not in _CACHE:
        _CACHE["nc"] = _build()
    return _CACHE["nc"]


def _host_prep(query, key, value, query_mask, key_mask, Wq, Wk, Wv, Wo):
    """Build the 8 per-core input maps (numpy only)."""
    inv = np.float32(1.0) / np.sqrt(np.float32(D))

    import ml_dtypes

    def tfeat(x):  # (B,S,D) -> feature-major (D, B*S), contiguous bf16
        return np.ascontiguousarray(
            x.reshape(B * S, D).astype(np.float32, copy=False).T
        ).astype(ml_dtypes.bfloat16)

    xq, xk, xv = tfeat(query), tfeat(key), tfeat(value)
    kmf = key_mask.astype(np.float32)
    qmf = query_mask.astype(np.float32)
    causal = np.triu(np.full((S, S), NEG, np.float32), k=1)
    Wqf = Wq.astype(np.float32, copy=False)
    Wkf = Wk.astype(np.float32, copy=False)
    Wvf = Wv.astype(np.float32, copy=False)
    Wof = Wo.astype(np.float32, copy=False)

    wo_p = np.stack(
        [
            np.concatenate(
                [Wof[p * DH:(p + 1) * DH, :], Wof[(p + 4) * DH:(p + 5) * DH, :]],
                axis=0,
            )
            for p in range(NPAIR)
        ]
    ).astype(ml_dtypes.bfloat16)  # (4, 128, 512)

    in_maps = []
    for a in range(H):
        madd_sum = (causal + NEG * (1.0 - kmf[a])[None, :]).astype(np.float32)
        # min-clip tile: exact masked values where masked, +FLT_MAX where not
        madd = np.where(madd_sum > 0, -madd_sum, FBIG).astype(np.float32)
        in_maps.append(
            {
                "xt_q": xq,
                "xt_k": xk,
                "xt_v": xv,
                "wq": np.ascontiguousarray(
                    Wqf[:, a * DH:(a + 1) * DH] * inv
                ).astype(ml_dtypes.bfloat16),
                "wk": np.ascontiguousarray(
                    Wkf[:, a * DH:(a + 1) * DH]
                ).astype(ml_dtypes.bfloat16),
                "wv": np.ascontiguousarray(
                    Wvf[:, a * DH:(a + 1) * DH]
                ).astype(ml_dtypes.bfloat16),
                "wo_p": wo_p,
                "madd": madd,
                "kmb": np.ascontiguousarray(
                    np.broadcast_to(kmf[a][None, :], (128, S))
                ).astype(ml_dtypes.bfloat16),
                "n1t": np.ascontiguousarray(
                    np.broadcast_to(
                        np.array(
                            [kmf[a, 128 * (i + 1):].sum() for i in range(NBLK)],
                            np.float32,
                        )[None, :],
                        (128, NBLK),
                    )
                ),
                "qmask": np.ascontiguousarray(
                    qmf[a].reshape(NBLK, 128).T
                ),  # [p, blk] = qm[a, 128*blk + p]
            }
        )
    return in_maps


def kernel(**inputs) -> np.ndarray:
    from concourse.bass_utils import run_bass_kernel_spmd

    nc = _get_nc()
    in_maps = _host_prep(
        np.asarray(inputs["query"]),
        np.asarray(inputs["key"]),
        np.asarray(inputs["value"]),
        np.asarray(inputs["query_mask"]),
        np.asarray(inputs["key_mask"]),
        np.asarray(inputs["Wq"]),
        np.asarray(inputs["Wk"]),
        np.asarray(inputs["Wv"]),
        np.asarray(inputs["Wo"]),
    )
    res = run_bass_kernel_spmd(nc, in_maps, core_ids=list(range(H)), **RUN_KWARGS)
    global LAST_RESULT
    LAST_RESULT = res
    return np.stack([res.results[a]["out"] for a in range(H)])


# revision 29
# speedup vs baseline: 2.2308x; 1.0885x over previous
"""Trainium2 Bass kernel for nn_MultiHeadAttention_61778809586301.

Head-sharded across 8 NeuronCores: core `a` computes output row-group `a`
(which, per the reference's faithful-TF recombination, is attention head `a`
across all 8 batches, concatenated batch-major along channels, then Wo+relu).

v2: all transposes moved off the DMA XBAR (was 422us serialized on Sync)
onto the PE array; mask-clip+rowmax fused into one DVE op; PV matmuls
grouped by key-block j (shares LDWEIGHTS, exploits PSUM zero-region
semantics for the triangular accumulation); fully-masked-row tail applied
as rank-1 PE matmuls accumulated straight into the PV PSUM.

Per-core device work (all f32 accumulate, bf16 operands):
  - projections q/k/v for all 8 batches, head slice `a` (weights host-sliced,
    inputs host-transposed feature-major so contraction lands on partitions)
  - causal + key-mask softmax attention, exact additive-mask semantics
    (min-clip against precomputed f32-absorbed mask values)
  - recombine via sum_c O_c @ Wo[c-slot] with relu + query-mask epilogue

Batches are processed in pairs (c, c+4) packed into SBUF partition halves so
K=64 / M=64 matmuls run as concurrent row/col-tiled pairs on the PE array.
"""
import sys

if "/opt/trn_rl_repo" not in sys.path:
    sys.path.insert(0, "/opt/trn_rl_repo")

import numpy as np

B, S, D, H, DH = 8, 1024, 512, 8, 64
NEG = np.float32(1.0e9)
NPAIR = 4          # batch pairs (p, p+4)
NBLK = S // 128    # 8 sq blocks of 128
KO = D // 128      # 4 contraction chunks of 128
FBIG = float(np.finfo(np.float32).max)

_CACHE: dict = {}
RUN_KWARGS: dict = {}   # extra kwargs for run_bass_kernel_spmd (e.g. trace)
LAST_RESULT = None      # BassKernelResults of the most recent kernel() call


def _build():
    import os
    import concourse.mybir as mybir
    import concourse.tile as tile
    from concourse import bacc
    from concourse.masks import make_identity

    V_TAIL = int(os.environ.get("V_TAIL", "1"))      # rank-1 tail matmuls
    V_BATCHT = int(os.environ.get("V_BATCHT", "1"))  # batched PE transposes
    V_GROUPPV = int(os.environ.get("V_GROUPPV", "1"))  # j-grouped PV

    f32 = mybir.dt.float32
    bf16 = mybir.dt.bfloat16
    AX = mybir.AxisListType.X
    OP = mybir.AluOpType
    ACT = mybir.ActivationFunctionType
    nc = bacc.Bacc(
        "TRN2",
        target_bir_lowering=False,
        debug=False,
        enable_asserts=False,
        num_devices=H,
    )

    xt_q = nc.dram_tensor("xt_q", [D, B * S], bf16, kind="ExternalInput")
    xt_k = nc.dram_tensor("xt_k", [D, B * S], bf16, kind="ExternalInput")
    xt_v = nc.dram_tensor("xt_v", [D, B * S], bf16, kind="ExternalInput")
    wq_d = nc.dram_tensor("wq", [D, DH], bf16, kind="ExternalInput")
    wk_d = nc.dram_tensor("wk", [D, DH], bf16, kind="ExternalInput")
    wv_d = nc.dram_tensor("wv", [D, DH], bf16, kind="ExternalInput")
    wo_d = nc.dram_tensor("wo_p", [NPAIR, 128, D], bf16, kind="ExternalInput")
    madd_d = nc.dram_tensor("madd", [S, S], f32, kind="ExternalInput")
    kmb_d = nc.dram_tensor("kmb", [128, S], bf16, kind="ExternalInput")
    n1t_d = nc.dram_tensor("n1t", [128, NBLK], f32, kind="ExternalInput")
    qm_d = nc.dram_tensor("qmask", [128, NBLK], f32, kind="ExternalInput")
    out_d = nc.dram_tensor("out", [S, D], f32, kind="ExternalOutput")

    with tile.TileContext(nc) as tc:
        with (
            tc.tile_pool(name="fixed", bufs=1) as fixed,
            tc.tile_pool(name="stage", bufs=3) as stage,
            tc.tile_pool(name="proj", bufs=2) as proj,
            tc.tile_pool(name="work", bufs=3) as work,
            tc.tile_pool(name="stats", bufs=6) as stats,
            tc.tile_pool(name="psA", bufs=3, space="PSUM") as psA,
            tc.tile_pool(name="psT", bufs=3, space="PSUM") as psT,
            tc.tile_pool(name="psPV", bufs=1, space="PSUM") as psPV,
        ):
            # ---- constants / weights ----
            ident = fixed.tile([128, 128], f32, tag="ident")
            make_identity(nc, ident[:])
            ident_bf = fixed.tile([128, 128], bf16, tag="identbf")
            nc.vector.tensor_copy(ident_bf[:], ident[:])

            wq_sb = fixed.tile([128, KO, DH], bf16, tag="wq")
            wk_sb = fixed.tile([128, KO, DH], bf16, tag="wk")
            wv_sb = fixed.tile([128, KO, DH], bf16, tag="wv")
            nc.sync.dma_start(wq_sb[:], wq_d.rearrange("(ko ki) m -> ki ko m", ki=128))
            nc.sync.dma_start(wk_sb[:], wk_d.rearrange("(ko ki) m -> ki ko m", ki=128))
            nc.sync.dma_start(wv_sb[:], wv_d.rearrange("(ko ki) m -> ki ko m", ki=128))

            wo_sb = fixed.tile([128, NPAIR, D], bf16, tag="wo")
            nc.sync.dma_start(wo_sb[:], wo_d.rearrange("p ki n -> ki p n"))

            qm_sb = fixed.tile([128, NBLK], f32, tag="qm")
            nc.sync.dma_start(qm_sb[:], qm_d[:, :])

            kmb_sb = fixed.tile([128, S], bf16, tag="kmb")
            nc.sync.dma_start(kmb_sb[:], kmb_d[:, :])
            n1t_sb = fixed.tile([128, NBLK], f32, tag="n1t")
            nc.sync.dma_start(n1t_sb[:], n1t_d[:, :])
            neg_big = fixed.tile([128, 1], f32, tag="negbig")
            nc.vector.memset(neg_big[:], -1.0e9)

            madd_sb = []
            for i in range(NBLK):
                L = 128 * (i + 1)
                t = fixed.tile([128, L], f32, tag=f"madd{i}")
                nc.sync.dma_start(t[:], madd_d[128 * i:128 * (i + 1), :L])
                madd_sb.append(t)

            # persistent attention outputs, transposed: [dh of c | dh of c+4] x S
            ot_sb = [
                fixed.tile([128, S], bf16, tag=f"ot{p}", name=f"ot{p}")
                for p in range(NPAIR)
            ]
            # P^T block store: [j, i] grid of 128x128 bf16 blocks, per g.
            # Slot (j, i) holds P^T of score block (sq=i, sk=j); i<j unused.
            ptg = [
                fixed.tile([128, NBLK, NBLK, 128], bf16, tag=f"ptg{g}",
                           name=f"ptg{g}")
                for g in range(2)
            ]

            def emit_proj(p):
                """Projections + masked-V suffix sums + V-natural for pair p."""
                qkv_pair = []
                for name, xt, w_sb in (
                    ("q", xt_q, wq_sb), ("k", xt_k, wk_sb), ("v", xt_v, wv_sb),
                ):
                    pair_t = proj.tile([128, S], bf16, tag=f"{name}T",
                                       name=f"{name}T{p}")
                    for g in range(2):
                        c = p + 4 * g
                        st = stage.tile([128, KO, S], bf16, tag="xstage",
                                        name=f"st{p}{name}{g}")
                        nc.sync.dma_start(
                            st[:],
                            xt[:, c * S:(c + 1) * S].rearrange(
                                "(ko ki) s -> ki ko s", ki=128
                            ),
                        )
                        for hh in range(2):
                            ps = psA.tile([128, 512], f32, tag="ps",
                                          name=f"psp{p}{name}{g}{hh}")
                            for ko in range(KO):
                                nc.tensor.matmul(
                                    ps[64 * g:64 * (g + 1), :],
                                    lhsT=w_sb[:, ko, :],
                                    rhs=st[:, ko, 512 * hh:512 * (hh + 1)],
                                    start=(ko == 0),
                                    stop=(ko == KO - 1),
                                )
                            eng = nc.scalar if (g + hh) % 2 == 0 else None
                            if eng is None:
                                nc.vector.tensor_copy(
                                    pair_t[64 * g:64 * (g + 1),
                                           512 * hh:512 * (hh + 1)],
                                    ps[64 * g:64 * (g + 1), :],
                                )
                            else:
                                nc.scalar.activation(
                                    pair_t[64 * g:64 * (g + 1),
                                           512 * hh:512 * (hh + 1)],
                                    ps[64 * g:64 * (g + 1), :],
                                    ACT.Identity,
                                    bias=0.0,
                                    scale=1.0,
                                )
                    qkv_pair.append(pair_t)
                qT, kT, vT = qkv_pair

                # masked-V suffix sums: vks[:, i] = sum_{j>=128(i+1), km=1} v[j]
                vks = proj.tile([128, NBLK], f32, tag="vks", name=f"vks{p}")
                bsum = proj.tile([128, NBLK], f32, tag="bsum", name=f"bsum{p}")
                ttscr = work.tile([128, 128], f32, tag="ttscr",
                                  name=f"tts{p}")
                for b in range(NBLK):
                    nc.vector.scalar_tensor_tensor(
                        out=ttscr[:],
                        in0=vT[:, 128 * b:128 * (b + 1)],
                        scalar=1.0,
                        in1=kmb_sb[:, 128 * b:128 * (b + 1)],
                        op0=OP.mult,
                        op1=OP.mult,
                        accum_out=bsum[:, b:b + 1],
                    )
                nc.vector.memset(vks[:, NBLK - 1:NBLK], 0.0)
                for b in range(NBLK - 2, -1, -1):
                    nc.vector.tensor_tensor(
                        vks[:, b:b + 1], vks[:, b + 1:b + 2],
                        bsum[:, b + 1:b + 2], OP.add,
                    )
                # single-column PE transposes: vksT[0, i, :] = vks[:, i]^T,
                # all rows on partition 0 (tail matmuls keep base quadrant 0)
                vksT = proj.tile([128, NBLK, 128], bf16, tag="vksT",
                                 name=f"vksT{p}")
                for half in range(2):
                    pst = psA.tile([128, 512], f32, tag="ps",
                                   name=f"vksT{p}h{half}")
                    for t in range(4):
                        i = 4 * half + t
                        nc.tensor.matmul(
                            pst[0:1, 128 * t:128 * (t + 1)],
                            lhsT=vks[:, i:i + 1],
                            rhs=ident[:],
                            is_transpose=True,
                            start=(t == 0),
                            stop=(t == 3),
                        )
                    nc.vector.tensor_copy(
                        vksT[0:1, 4 * half:4 * half + 4, :],
                        pst[0:1, 0:512],
                    )
                # V back to natural layout [sk, dh] per (block j, g) via PE
                # transpose, batched 16 blocks into one PSUM bank.
                vnat = proj.tile([128, 2, NBLK, DH], bf16, tag="vnat",
                                 name=f"vnat{p}")
                for g in range(2):
                    psv = psT.tile([128, 1024], bf16, tag="tp1024",
                                   name=f"psv{p}{g}")
                    for j in range(NBLK):
                        nc.tensor.matmul(
                            psv[:, 64 * j:64 * (j + 1)],
                            lhsT=vT[64 * g:64 * (g + 1),
                                    128 * j:128 * (j + 1)],
                            rhs=ident_bf[64 * g:64 * (g + 1),
                                         64 * g:64 * (g + 1)],
                            is_transpose=True,
                            start=(j == 0),
                            stop=(j == NBLK - 1),
                        )
                    nc.scalar.activation(
                        vnat[:, g],
                        psv[:, 0:512],
                        ACT.Identity,
                        bias=0.0,
                        scale=1.0,
                    )
                return qT, kT, vT, vksT, vnat

            def emit_attn(p, tiles):
                qT, kT, vT, vksT, vnat = tiles
                ptailT = [None, None]  # [g]
                for g in range(2):
                    ptail_all = stats.tile([128, NBLK], f32, tag="ptall",
                                           name=f"ptall{p}{g}")
                    for i in range(NBLK):
                        L = 128 * (i + 1)
                        nch = (L + 511) // 512
                        t_sb = work.tile([128, S], f32, tag="tneg")
                        for n in range(nch):
                            Ln = min(512, L - 512 * n)
                            ps = psA.tile([128, 512], f32, tag="ps")
                            nc.tensor.matmul(
                                ps[:, :Ln],
                                lhsT=qT[64 * g:64 * (g + 1),
                                        128 * i:128 * (i + 1)],
                                rhs=kT[64 * g:64 * (g + 1),
                                       512 * n:512 * n + Ln],
                                start=True,
                                stop=True,
                            )
                            # t = min(sims, mclip): masked entries become the
                            # exact post-mask constants (-1e9/-2e9, matching
                            # jax's f32 absorption of "x - 1e9").
                            nc.vector.tensor_tensor(
                                t_sb[:, 512 * n:512 * n + Ln],
                                ps[:, :Ln],
                                madd_sb[i][:, 512 * n:512 * n + Ln],
                                OP.min,
                            )
                        mneg = stats.tile([128, 1], f32, tag="mneg")
                        nc.vector.tensor_reduce(
                            mneg[:],
                            t_sb[:, :L],
                            axis=AX,
                            op=OP.max,
                            negate=True,
                        )
                        p_sb = work.tile([128, S], f32, tag="p")
                        ssum = stats.tile([128, 1], f32, tag="ssum")
                        nc.scalar.activation(
                            p_sb[:, :L],
                            t_sb[:, :L],
                            ACT.Exp,
                            bias=mneg[:],
                            scale=1.0,
                            accum_out=ssum[:],
                        )
                        # virtual tail column: weight exp(-1e9 - m) per row
                        # (0 for normal rows; 1 for fully-masked rows), with
                        # n1t tail-tie count folded into the softmax sum.
                        etail = stats.tile([128, 1], f32, tag="etail")
                        nc.scalar.activation(
                            etail[:],
                            neg_big[:],
                            ACT.Exp,
                            bias=mneg[:],
                            scale=1.0,
                        )
                        ssum2 = stats.tile([128, 1], f32, tag="ssum2")
                        nc.vector.scalar_tensor_tensor(
                            out=ssum2[:],
                            in0=etail[:],
                            scalar=n1t_sb[:, i:i + 1],
                            in1=ssum[:],
                            op0=OP.mult,
                            op1=OP.add,
                        )
                        rcp = stats.tile([128, 1], f32, tag="rcp")
                        nc.vector.reciprocal(rcp[:], ssum2[:])
                        nc.vector.tensor_tensor(
                            ptail_all[:, i:i + 1],
                            etail[:], rcp[:], OP.mult,
                        )
                        pn_sb = work.tile([128, S], bf16, tag="pn")
                        nc.scalar.activation(
                            pn_sb[:, :L],
                            p_sb[:, :L],
                            ACT.Identity,
                            bias=0.0,
                            scale=rcp[:],
                        )
                        # PE-transpose the (i+1) P blocks into one PSUM bank,
                        # then one batched copy into the [j, i] grid.
                        if V_BATCHT:
                            pst = psT.tile([128, 1024], bf16, tag="tp1024")
                            for j in range(i + 1):
                                nc.tensor.matmul(
                                    pst[:, 128 * j:128 * (j + 1)],
                                    lhsT=pn_sb[:, 128 * j:128 * (j + 1)],
                                    rhs=ident_bf[:],
                                    is_transpose=True,
                                    start=(j == 0),
                                    stop=(j == i),
                                )
                            dst = ptg[g][:, 0:i + 1, i, :]
                            if i % 2 == 0:
                                nc.vector.tensor_copy(dst, pst[:, :L])
                            else:
                                nc.scalar.activation(
                                    dst, pst[:, :L], ACT.Identity,
                                    bias=0.0, scale=1.0,
                                )
                        else:
                            for j in range(i + 1):
                                pst = psT.tile([128, 1024], bf16,
                                               tag="tp1024")
                                nc.tensor.matmul(
                                    pst[:, 0:128],
                                    lhsT=pn_sb[:, 128 * j:128 * (j + 1)],
                                    rhs=ident_bf[:],
                                    is_transpose=True,
                                    start=True,
                                    stop=True,
                                )
                                dst = ptg[g][:, j, i, :]
                                if (i + j) % 2 == 0:
                                    nc.vector.tensor_copy(dst, pst[:, 0:128])
                                else:
                                    nc.scalar.activation(
                                        dst, pst[:, 0:128], ACT.Identity,
                                        bias=0.0, scale=1.0,
                                    )
                    # single-column transposes: ptailT[g][0, i, :] =
                    # ptail_i^T, all rows on partition 0
                    ptT = stats.tile([128, NBLK, 128], bf16, tag="ptTs",
                                     name=f"ptTs{p}{g}")
                    for half in range(2):
                        pst = psA.tile([128, 512], f32, tag="ps",
                                       name=f"ptT{p}{g}h{half}")
                        for t in range(4):
                            i = 4 * half + t
                            nc.tensor.matmul(
                                pst[0:1, 128 * t:128 * (t + 1)],
                                lhsT=ptail_all[:, i:i + 1],
                                rhs=ident[:],
                                is_transpose=True,
                                start=(t == 0),
                                stop=(t == 3),
                            )
                        nc.vector.tensor_copy(
                            ptT[0:1, 4 * half:4 * half + 4, :],
                            pst[0:1, 0:512],
                        )
                    ptailT[g] = ptT

                # PV grouped by key block j: lhsT (V natural) loaded once per
                # (j, g); triangular column extents ride on PSUM zero-region
                # semantics (j==0 start zeroes the full bank).
                ps_pv = psPV.tile([128, 1024], f32, tag="pv", name=f"pv{p}")
                for g in range(2):
                    gs = slice(64 * g, 64 * (g + 1))
                    if V_GROUPPV:
                        for j in range(NBLK):
                            if j < 4:
                                nc.tensor.matmul(
                                    ps_pv[gs, 128 * j:512],
                                    lhsT=vnat[:, g, j, :],
                                    rhs=ptg[g][:, j, j:4, :],
                                    start=(j == 0),
                                    stop=(j == 3),
                                )
                                nc.tensor.matmul(
                                    ps_pv[gs, 512:1024],
                                    lhsT=vnat[:, g, j, :],
                                    rhs=ptg[g][:, j, 4:8, :],
                                    start=(j == 0),
                                    stop=False,
                                )
                            else:
                                nc.tensor.matmul(
                                    ps_pv[gs, 128 * j:1024],
                                    lhsT=vnat[:, g, j, :],
                                    rhs=ptg[g][:, j, j:8, :],
                                    start=False,
                                    stop=(j == NBLK - 1),
                                )
                    else:
                        for i in range(NBLK):
                            for j in range(i + 1):
                                nc.tensor.matmul(
                                    ps_pv[gs, 128 * i:128 * (i + 1)],
                                    lhsT=vnat[:, g, j, :],
                                    rhs=ptg[g][:, j, i, :],
                                    start=(j == 0),
                                    stop=(not V_TAIL and j == i),
                                    skip_group_check=True,
                                )
                    # tail: rank-1 ptail[sq]*vks[dh] per block, base-0 K=1
                    # matmuls continuing the closed PV accumulation (fixed
                    # quadrant; rotation inside an accumulation chain faults)
                    for i in range(NBLK):
                        nc.tensor.matmul(
                            ps_pv[gs, 128 * i:128 * (i + 1)],
                            lhsT=vksT[0:1, i, 64 * g:64 * (g + 1)],
                            rhs=ptailT[g][0:1, i, :],
                            start=False,
                            stop=True,
                            skip_group_check=True,
                        )
                    # drain attention output (normalized, tail included)
                    nc.vector.tensor_copy(
                        ot_sb[p][gs, 0:512], ps_pv[gs, 0:512],
                    )
                    nc.scalar.activation(
                        ot_sb[p][gs, 512:1024], ps_pv[gs, 512:1024],
                        ACT.Identity, bias=0.0, scale=1.0,
                    )

            # ---- software-pipelined emission: proj one pair ahead ----
            tiles = emit_proj(0)
            nxt = emit_proj(1)
            emit_attn(0, tiles)
            tiles, nxt = nxt, emit_proj(2)
            emit_attn(1, tiles)
            tiles, nxt = nxt, emit_proj(3)
            emit_attn(2, tiles)
            emit_attn(3, nxt)

            # ---- final projection + relu + query-mask ----
            for i in range(NBLK):
                ps = psA.tile([128, 512], f32, tag="ps", name=f"psf{i}")
                for p in range(NPAIR):
                    nc.tensor.matmul(
                        ps[:],
                        lhsT=ot_sb[p][:, 128 * i:128 * (i + 1)],
                        rhs=wo_sb[:, p, :],
                        start=(p == 0),
                        stop=(p == NPAIR - 1),
                    )
                o_sb = work.tile([128, D], f32, tag="osb")
                nc.scalar.activation(
                    o_sb[:],
                    ps[:],
                    ACT.Relu,
                    bias=0.0,
                    scale=qm_sb[:, i:i + 1],
                )
                nc.sync.dma_start(out_d[128 * i:128 * (i + 1), :], o_sb[:])

    nc.compile()
    return nc


def _get_nc():
    if "nc" not in _CACHE:
        _CACHE["nc"] = _build()
    return _CACHE["nc"]


def _host_prep(query, key, value, query_mask, key_mask, Wq, Wk, Wv, Wo):
    """Build the 8 per-core input maps (numpy only)."""
    inv = np.float32(1.0) / np.sqrt(np.float32(D))

    import ml_dtypes

    def tfeat(x):  # (B,S,D) -> feature-major (D, B*S), contiguous bf16
        return np.ascontiguousarray(
            x.reshape(B * S, D).astype(np.float32, copy=False).T
        ).astype(ml_dtypes.bfloat16)

    xq, xk, xv = tfeat(query), tfeat(key), tfeat(value)
    kmf = key_mask.astype(np.float32)
    qmf = query_mask.astype(np.float32)
    causal = np.triu(np.full((S, S), NEG, np.float32), k=1)
    Wqf = Wq.astype(np.float32, copy=False)
    Wkf = Wk.astype(np.float32, copy=False)
    Wvf = Wv.astype(np.float32, copy=False)
    Wof = Wo.astype(np.float32, copy=False)

    wo_p = np.stack(
        [
            np.concatenate(
                [Wof[p * DH:(p + 1) * DH, :], Wof[(p + 4) * DH:(p + 5) * DH, :]],
                axis=0,
            )
            for p in range(NPAIR)
        ]
    ).astype(ml_dtypes.bfloat16)  # (4, 128, 512)

    in_maps = []
    for a in range(H):
        madd_sum = (causal + NEG * (1.0 - kmf[a])[None, :]).astype(np.float32)
        # min-clip tile: exact masked values where masked, +FLT_MAX where not
        madd = np.where(madd_sum > 0, -madd_sum, FBIG).astype(np.float32)
        in_maps.append(
            {
                "xt_q": xq,
                "xt_k": xk,
                "xt_v": xv,
                "wq": np.ascontiguousarray(
                    Wqf[:, a * DH:(a + 1) * DH] * inv
                ).astype(ml_dtypes.bfloat16),
                "wk": np.ascontiguousarray(
                    Wkf[:, a * DH:(a + 1) * DH]
                ).astype(ml_dtypes.bfloat16),
                "wv": np.ascontiguousarray(
                    Wvf[:, a * DH:(a + 1) * DH]
                ).astype(ml_dtypes.bfloat16),
                "wo_p": wo_p,
                "madd": madd,
                "kmb": np.ascontiguousarray(
                    np.broadcast_to(kmf[a][None, :], (128, S))
                ).astype(ml_dtypes.bfloat16),
                "n1t": np.ascontiguousarray(
                    np.broadcast_to(
                        np.array(
                            [kmf[a, 128 * (i + 1):].sum() for i in range(NBLK)],
                            np.float32,
                        )[None, :],
                        (128, NBLK),
                    )
                ),
                "qmask": np.ascontiguousarray(
                    qmf[a].reshape(NBLK, 128).T
                ),  # [p, blk] = qm[a, 128*blk + p]
            }
        )
    return in_maps


def kernel(**inputs) -> np.ndarray:
    from concourse.bass_utils import run_bass_kernel_spmd

    nc = _get_nc()
    in_maps = _host_prep(
        np.asarray(inputs["query"]),
        np.asarray(inputs["key"]),
        np.asarray(inputs["value"]),
        np.asarray(inputs["query_mask"]),
        np.asarray(inputs["key_mask"]),
        np.asarray(inputs["Wq"]),
        np.asarray(inputs["Wk"]),
        np.asarray(inputs["Wv"]),
        np.asarray(inputs["Wo"]),
    )
    res = run_bass_kernel_spmd(nc, in_maps, core_ids=list(range(H)), **RUN_KWARGS)
    global LAST_RESULT
    LAST_RESULT = res
    return np.stack([res.results[a]["out"] for a in range(H)])
